# revision 1
# baseline (speedup 1.0000x reference)
"""Trainium2 Bass kernel for nn_ConcatHeadModule (pairwise concat-head scorer).

Math (reference):
    xc   = x.reshape(T, 2L)
    actH = tanh(xc @ W1H + cbH);  actM = tanh(xc @ W1M + cbM)
    AH   = actH @ L2H;            AM   = actM @ L2M
    scores[i,j] = sum_h wOut[h]*tanh(AH[i,h] + AM[j,h] + h2b[h]) + outBias

Sharding: row-shard the [T,T] score grid across 8 cores (96 rows each).
Each core builds the full AM^T (h-major) once, then for each of its 96
rows i evaluates tanh(AM^T[h, j] + (AH[i,h]+h2b[h])) with the pairwise
add fused into ScalarE's per-partition activation bias, and contracts
over h with TensorE (bf16) into a [1, 768] PSUM row.

All shapes are hardcoded (T=768, 2L=512, HID=512, HID2=512, 8 cores).
"""

import os
import sys

for _p in ("/root/.axon_site", "/root/.axon_site/_ro/trn_rl_repo", "/opt/trn_rl_repo"):
    if os.path.isdir(_p) and _p not in sys.path:
        sys.path.append(_p)

import ml_dtypes
import numpy as np

import concourse.bass as bass
import concourse.mybir as mybir
import concourse.tile as tile
from concourse import bacc
from concourse.bass_utils import run_bass_kernel_spmd

F32 = mybir.dt.float32
F32R = mybir.dt.float32r
BF16 = mybir.dt.bfloat16
TANH = mybir.ActivationFunctionType.Tanh

T = 768          # tokens
C = 512          # 2 * LDIMS (concat lstm state)
H = 512          # hidden1
H2 = 512         # hidden2
NCORES = 8
R = T // NCORES  # score rows per core
P = 128          # partitions
NKC = C // P     # contraction chunks over C
NKH = H // P     # chunks over H
NKH2 = H2 // P   # chunks over H2


def build_nc(rows: int = R) -> bass.Bass:
    nc = bacc.Bacc("TRN2", target_bir_lowering=False, num_devices=NCORES)

    xT = nc.dram_tensor("xT", [C, T], F32R, kind="ExternalInput")
    xTi = nc.dram_tensor("xTi", [C, rows], F32, kind="ExternalInput")
    w1h = nc.dram_tensor("w1h", [C, H], F32, kind="ExternalInput")
    w1m = nc.dram_tensor("w1m", [C, H], F32R, kind="ExternalInput")
    hid2h = nc.dram_tensor("hid2h", [H, H2], F32, kind="ExternalInput")
    hid2m = nc.dram_tensor("hid2m", [H, H2], F32R, kind="ExternalInput")
    cbh = nc.dram_tensor("cbh", [P, NKH], F32, kind="ExternalInput")
    cbm = nc.dram_tensor("cbm", [P, NKH], F32, kind="ExternalInput")
    h2b = nc.dram_tensor("h2b", [P, NKH2], F32, kind="ExternalInput")
    wout = nc.dram_tensor("wout", [P, NKH2 * 32], BF16, kind="ExternalInput")
    ob = nc.dram_tensor("ob", [P, 1], F32, kind="ExternalInput")
    out_rows = nc.dram_tensor("out_rows", [rows, T], F32, kind="ExternalOutput")

    with tile.TileContext(nc) as tc:
        _emit(tc, locals(), rows)
    nc.compile()
    return nc


def _emit(tc: tile.TileContext, io, rows: int):
    nc = tc.nc
    xT, xTi, w1h, w1m = io["xT"], io["xTi"], io["w1h"], io["w1m"]
    hid2h, hid2m = io["hid2h"], io["hid2m"]
    cbh, cbm, h2b, wout, ob = io["cbh"], io["cbm"], io["h2b"], io["wout"], io["ob"]
    out_rows = io["out_rows"]

    with tc.tile_pool(name="const", bufs=1) as const:
        setup_pool_cm = tc.tile_pool(name="setup_sb", bufs=1)
        setup = setup_pool_cm.__enter__()
        # ---- load inputs ----
        # One coalesced DMA per tensor (SP-queue issue is ~0.65us per DMA,
        # so 8 big DMAs beat ~40 small ones), ordered by first use: biases,
        # then the M-side chain that gates the first activations, then the
        # H side.
        def load_coalesced(pool, name, dram, k, inner, dt, eng=None):
            t = pool.tile([P, k * inner], dt, name=name)
            (eng or nc.sync).dma_start(
                t[:].rearrange("p (k t) -> p k t", k=k),
                dram[:].rearrange("(k p) t -> p k t", p=P),
            )
            return t, [t[:, j * inner:(j + 1) * inner] for j in range(k)]

        cb_all = const.tile([P, 2 * NKH + NKH2 + 1], F32, name="cb_all")
        nc.sync.dma_start(cb_all[:, 0:NKH], cbm[:, :])
        nc.sync.dma_start(cb_all[:, NKH:2 * NKH], cbh[:, :])
        nc.sync.dma_start(cb_all[:, 2 * NKH:2 * NKH + NKH2], h2b[:, :])
        nc.sync.dma_start(cb_all[:, 2 * NKH + NKH2:], ob[:, :])
        cbm_sb = [cb_all[:, k:k + 1] for k in range(NKH)]
        cbh_sb = [cb_all[:, NKH + k:NKH + k + 1] for k in range(NKH)]
        h2b_sb = [cb_all[:, 2 * NKH + k:2 * NKH + k + 1] for k in range(NKH2)]
        ob_sb = cb_all[:, 2 * NKH + NKH2:2 * NKH + NKH2 + 1]
        wout_all = const.tile([P, NKH2 * 32], BF16, name="wout_all")
        nc.sync.dma_start(wout_all[:], wout[:, :])
        wout_sb = [wout_all[:, 32 * k:32 * (k + 1)] for k in range(NKH2)]

        # chunked so the first Mf^T chain starts as soon as chunk 0 lands;
        # alternate the two HWDGE queues (SP + ACT) for parallel transfer
        w1m_all = setup.tile([P, NKC * H], F32R, name="w1m_all")
        xT_all = setup.tile([P, NKC * T], F32R, name="xT_all")
        w1m_sb = []
        xT_sb = []
        for k in range(NKC):
            nc.scalar.dma_start(w1m_all[:, k * H:(k + 1) * H],
                                w1m[k * P:(k + 1) * P, :])
            w1m_sb.append(w1m_all[:, k * H:(k + 1) * H])
            nc.sync.dma_start(xT_all[:, k * T:(k + 1) * T],
                              xT[k * P:(k + 1) * P, :])
            xT_sb.append(xT_all[:, k * T:(k + 1) * T])
        _, l2m_sb = load_coalesced(setup, "l2m_all", hid2m, NKH, H2, F32R)
        _, xTi_sb = load_coalesced(setup, "xTi_all", xTi, NKC, rows, F32)
        _, w1h_sb = load_coalesced(setup, "w1h_all", w1h, NKC, H, F32)
        _, l2h_sb = load_coalesced(setup, "l2h_all", hid2h, NKH, H2, F32)

        # ---- setup: actM^T = tanh(Mf^T + cbm), Mf^T = W1M^T @ xc^T ----
        NHALF = T // 2  # 384 <= 512 fp32 moving-operand limit
        setup_ps_cm = tc.tile_pool(name="setup_ps", bufs=4, space="PSUM")
        setup_ps = setup_ps_cm.__enter__()
        actMT = []
        for hc in range(NKH):
            amt = setup.tile([P, T], F32R, name=f"actMT{hc}")
            for n0 in range(0, T, NHALF):
                ps = setup_ps.tile([P, NHALF], F32, tag="setup")
                for cc in range(NKC):
                    nc.tensor.matmul(
                        ps[:],
                        lhsT=w1m_sb[cc][:, hc * P:(hc + 1) * P],
                        rhs=xT_sb[cc][:, n0:n0 + NHALF],
                        start=(cc == 0),
                        stop=(cc == NKC - 1),
                    )
                nc.scalar.activation(
                    amt[:, n0:n0 + NHALF], ps[:], TANH, bias=cbm_sb[hc][:]
                )
            actMT.append(amt)

        # actH^T for this core's rows = tanh(Hf^T + cbh)
        actHT = []
        for hc in range(NKH):
            aht = setup.tile([P, rows], F32, name=f"actHT{hc}")
            ps = setup_ps.tile([P, rows], F32, tag="setup")
            for cc in range(NKC):
                nc.tensor.matmul(
                    ps[:],
                    lhsT=w1h_sb[cc][:, hc * P:(hc + 1) * P],
                    rhs=xTi_sb[cc][:],
                    start=(cc == 0),
                    stop=(cc == NKC - 1),
                )
            nc.scalar.activation(aht[:], ps[:], TANH, bias=cbh_sb[hc][:])
            actHT.append(aht)

        # AM^T = L2M^T @ actM^T  (stored bf16 so the DVE pairwise add runs 4x)
        AMT = []
        for hc in range(NKH2):
            am = const.tile([P, T], BF16, name=f"AMT{hc}")
            for n0 in range(0, T, NHALF):
                ps = setup_ps.tile([P, NHALF], F32, tag="setup")
                for kc in range(NKH):
                    nc.tensor.matmul(
                        ps[:],
                        lhsT=l2m_sb[kc][:, hc * P:(hc + 1) * P],
                        rhs=actMT[kc][:, n0:n0 + NHALF],
                        start=(kc == 0),
                        stop=(kc == NKH - 1),
                    )
                nc.vector.tensor_copy(am[:, n0:n0 + NHALF], ps[:])
            AMT.append(am)

        # ABIAS[h, i] = AH^T[h, i] + h2b[h]
        ABIAS = []
        for hc in range(NKH2):
            ab = const.tile([P, rows], F32, name=f"ABIAS{hc}")
            ps = setup_ps.tile([P, rows], F32, tag="setup")
            for kc in range(NKH):
                nc.tensor.matmul(
                    ps[:],
                    lhsT=l2h_sb[kc][:, hc * P:(hc + 1) * P],
                    rhs=actHT[kc][:],
                    start=(kc == 0),
                    stop=(kc == NKH - 1),
                )
            nc.vector.tensor_scalar_add(ab[:], ps[:], h2b_sb[hc][:])
            ABIAS.append(ab)

        setup_ps_cm.__exit__(None, None, None)
        setup_pool_cm.__exit__(None, None, None)

        # ---- main loop ----
        # Per group of G rows: DVE builds bf16 pairwise-sum tiles (4x mode),
        # one big-FD tanh per h-chunk on ScalarE, TensorE contracts against
        # wOut with 4 rows per PSUM tile via tile_position col groups.
        if rows % 12 == 0 and rows >= 24:
            group_sizes = [12] * (rows // 12 - 1) + [8, 4]
        elif rows % 8 == 0 and rows >= 16:
            group_sizes = [8] * (rows // 8 - 1) + [4, 4]
        elif rows % 4 == 0:
            group_sizes = [4] * (rows // 4)
        else:
            group_sizes = [1] * rows
        with (
            tc.tile_pool(name="spool", bufs=3) as spool,
            tc.tile_pool(name="zpool", bufs=NKH2 + 2) as zpool,
            tc.tile_pool(name="evbuf", bufs=3) as evpool,
            tc.tile_pool(name="row_ps", bufs=4, space="PSUM") as row_ps,
        ):
            i0 = 0
            for g, G in enumerate(group_sizes):
                if G % 12 == 0 or G % 8 == 0:
                    QR = 4
                elif G % 4 == 0:
                    QR = 2
                else:
                    QR = 1
                NQ = G // QR
                Zs = []
                for hc in range(NKH2):
                    S = spool.tile([P, G * T], BF16, tag="s", name=f"S{g}_{hc}")
                    for u in range(G):
                        nc.vector.tensor_scalar_add(
                            S[:, u * T:(u + 1) * T], AMT[hc][:],
                            ABIAS[hc][:, i0 + u:i0 + u + 1],
                        )
                    Z = zpool.tile([P, G * T], BF16, tag="z", name=f"Z{g}_{hc}")
                    nc.scalar.activation(Z[:], S[:], TANH)
                    Zs.append(Z)
                # Wave-scheduled contraction: wave w covers regions
                # (q, cg=(q+w)%QR) — pending PSUM accumulation groups sit in
                # disjoint banks (different q tiles), consecutive matmuls
                # rotate PE col strips so LDWEIGHTS overlaps in-flight
                # matmuls, and the stationary (wout[hc]) is constant across
                # each wave.
                psrs = [
                    row_ps.tile([P, T], F32, tag="row", name=f"psr{g}_{q}")
                    for q in range(NQ)
                ]
                for w in range(QR):
                    for hc in range(NKH2):
                        for n0, nw in ((0, 512), (512, T - 512)):
                            for q in range(NQ):
                                cg = (q + w) % QR
                                u = q * QR + cg
                                nc.tensor.matmul(
                                    psrs[q][32 * cg:32 * cg + 32, n0:n0 + nw],
                                    lhsT=wout_sb[hc][:],
                                    rhs=Zs[hc][:, u * T + n0:u * T + n0 + nw],
                                    start=(hc == 0),
                                    stop=(hc == NKH2 - 1),
                                    tile_position=(0, 32 * cg),
                                )
                for q in range(NQ):
                    ev = evpool.tile([P, T], F32, tag="ev", name=f"ev{g}_{q}")
                    nc.vector.tensor_scalar_add(
                        ev[0:32 * QR, :], psrs[q][0:32 * QR, :],
                        ob_sb[0:32 * QR, :],
                    )
                    for cg in range(QR):
                        i = i0 + q * QR + cg
                        nc.sync.dma_start(
                            out_rows[i:i + 1, :], ev[32 * cg:32 * cg + 1, :]
                        )
                i0 += G


def _prep_inputs(x, hidLayerFOH, hidLayerFOM, catBias, hid2Layer, hid2Bias,
                 outLayer, outBias, rows=R, ncores=NCORES):
    """Host-side layout prep (reshape/transpose/slice/cast only)."""
    x = np.asarray(x, np.float32)
    xc = x.reshape(T, C)
    xT_np = np.ascontiguousarray(xc.T)
    common = {
        "xT": xT_np,
        "w1h": np.ascontiguousarray(np.asarray(hidLayerFOH, np.float32)),
        "w1m": np.ascontiguousarray(np.asarray(hidLayerFOM, np.float32)),
        "hid2h": np.ascontiguousarray(np.asarray(hid2Layer, np.float32)[:H]),
        "hid2m": np.ascontiguousarray(np.asarray(hid2Layer, np.float32)[H:]),
        "cbh": np.ascontiguousarray(
            np.asarray(catBias[:H], np.float32).reshape(NKH, P).T),
        "cbm": np.ascontiguousarray(
            np.asarray(catBias[H:], np.float32).reshape(NKH, P).T),
        "h2b": np.ascontiguousarray(
            np.asarray(hid2Bias, np.float32).reshape(NKH2, P).T),
        "wout": np.ascontiguousarray(np.repeat(
            np.asarray(outLayer, np.float32).astype(ml_dtypes.bfloat16)
            .reshape(NKH2, P).T, 32, axis=1)),
        "ob": np.full((P, 1), np.asarray(outBias, np.float32).reshape(()),
                      np.float32),
    }
    in_maps = []
    for c in range(ncores):
        m = dict(common)
        m["xTi"] = np.ascontiguousarray(xc[c * rows:(c + 1) * rows].T)
        in_maps.append(m)
    return in_maps


def kernel(x, hidLayerFOH, hidLayerFOM, catBias, hid2Layer, hid2Bias,
           outLayer, outBias, _trace=False):
    in_maps = _prep_inputs(x, hidLayerFOH, hidLayerFOM, catBias,
                           hid2Layer, hid2Bias, outLayer, outBias)
    nc = build_nc(R)
    res = run_bass_kernel_spmd(nc, in_maps, core_ids=list(range(NCORES)),
                               trace=_trace)
    out = np.concatenate([res.results[c]["out_rows"] for c in range(NCORES)], 0)
    if _trace:
        kernel.last_results = res
    return out.astype(np.float32)



# revision 2
# speedup vs baseline: 2.8543x; 2.8543x over previous
"""Trainium2 Bass kernel for nn_ConcatHeadModule (pairwise concat-head scorer).

Math (reference):
    xc   = x.reshape(T, 2L)
    actH = tanh(xc @ W1H + cbH);  actM = tanh(xc @ W1M + cbM)
    AH   = actH @ L2H;            AM   = actM @ L2M
    scores[i,j] = sum_h wOut[h]*tanh(AH[i,h] + AM[j,h] + h2b[h]) + outBias

Instead of evaluating the T*T*H2 pairwise tanh on ScalarE (the baseline's
bottleneck: ~250us of ACTIVATE per core), tanh is expanded in an odd
harmonic sine series fitted offline (uniform error ~2e-3 on |s|<=4.8):

    tanh(s) ~= sum_k g_k * sin((2k-1)*w0*s),   k = 1..K

Each term is separable across the pairwise sum s = a_i + B_j:

    sin(w(a+B)) = sin(wa)cos(wB) + cos(wa)sin(wB)

so the score matrix becomes a single TensorE contraction over (k, h):

    scores[i,j] = sum_k sum_h [g_k*w_h*sin(w_k a)]*cos(w_k B)
                            + [g_k*w_h*cos(w_k a)]*sin(w_k B)

Only the base streams sin(w0*v), cos(w0*v), cos(2*w0*v) are evaluated with
ScalarE's Sin table (arguments stay inside its valid [-pi, pi] domain);
higher harmonics come from the 3-term Chebyshev-style recurrence
    s_{k+1} = 2*cos(2*w0*v)*s_k - s_{k-1}
on the vector engine in bf16 (sin/cos chains concatenated into one tile to
halve instruction count). Sharding: rows of the score grid, 96 per core.

All shapes are hardcoded (T=768, 2L=512, HID=512, HID2=512, 8 cores).
"""

import os
import sys

for _p in ("/root/.axon_site", "/root/.axon_site/_ro/trn_rl_repo", "/opt/trn_rl_repo"):
    if os.path.isdir(_p) and _p not in sys.path:
        sys.path.append(_p)

import math

import ml_dtypes
import numpy as np

import concourse.bass as bass
import concourse.mybir as mybir
import concourse.tile as tile
from concourse import bacc
from concourse.bass_utils import run_bass_kernel_spmd

F32 = mybir.dt.float32
BF16 = mybir.dt.bfloat16
TANH = mybir.ActivationFunctionType.Tanh
SIN = mybir.ActivationFunctionType.Sin
MULT = mybir.AluOpType.mult
ADD = mybir.AluOpType.add
SUB = mybir.AluOpType.subtract

T = 768          # tokens
C = 512          # 2 * LDIMS (concat lstm state)
H = 512          # hidden1
H2 = 512         # hidden2
NCORES = 8
R = T // NCORES  # score rows per core
P = 128          # partitions
NK = 4           # 128-chunks in C/H/H2
TH = T // 2      # 384, psum half-width for j

# Odd-harmonic sine expansion of tanh: tanh(s) ~= sum g[k] sin((2k+1) W0 s)
W0 = 0.32
GAMMA = [1.2211527, 0.29795354, 0.10435029, 0.037995062, 0.013862776,
         0.0058787093]
K = len(GAMMA)


def build_nc(rows: int = R) -> bass.Bass:
    nc = bacc.Bacc("TRN2", target_bir_lowering=False, num_devices=NCORES)

    xT = nc.dram_tensor("xT", [C, T], BF16, kind="ExternalInput")
    xTi = nc.dram_tensor("xTi", [C, rows], BF16, kind="ExternalInput")
    w1m = nc.dram_tensor("w1m", [C, H], BF16, kind="ExternalInput")
    w1h = nc.dram_tensor("w1h", [C, H], BF16, kind="ExternalInput")
    l2m = nc.dram_tensor("l2m", [H, H2], BF16, kind="ExternalInput")
    l2h = nc.dram_tensor("l2h", [H, H2], BF16, kind="ExternalInput")
    # cb_all columns: cbm[4], cbh[4], h2b[4], pihalf[1], ob[1]
    cb = nc.dram_tensor("cb", [P, 3 * NK + 2], F32, kind="ExternalInput")
    wexp = nc.dram_tensor("wexp", [P, 2 * NK * R], F32, kind="ExternalInput")
    out_rows = nc.dram_tensor("out_rows", [rows, T], F32, kind="ExternalOutput")

    with tile.TileContext(nc) as tc:
        _emit(tc, locals(), rows)
    nc.compile()
    return nc


def _emit(tc: tile.TileContext, io, rows: int):
    nc = tc.nc
    xT, xTi, w1m, w1h = io["xT"], io["xTi"], io["w1m"], io["w1h"]
    l2m, l2h, cb, wexp = io["l2m"], io["l2h"], io["cb"], io["wexp"]
    out_rows = io["out_rows"]
    UW = 2 * NK * rows  # 768: width of concatenated [sin|cos] U tiles

    with tc.tile_pool(name="const", bufs=1) as const:
        setup_cm = tc.tile_pool(name="setup_sb", bufs=1)
        setup = setup_cm.__enter__()

        # ---- input DMAs (few, coalesced; alternate the two HWDGE queues) ----
        cb_all = const.tile([P, 3 * NK + 2], F32, name="cb_all")
        nc.sync.dma_start(cb_all[:], cb[:, :])
        cbm_sb = [cb_all[:, k:k + 1] for k in range(NK)]
        cbh_sb = [cb_all[:, NK + k:NK + k + 1] for k in range(NK)]
        h2b_sb = [cb_all[:, 2 * NK + k:2 * NK + k + 1] for k in range(NK)]
        pihalf = cb_all[:, 3 * NK:3 * NK + 1]
        ob_sb = cb_all[:, 3 * NK + 1:3 * NK + 2]
        wexp_sb = const.tile([P, UW], F32, name="wexp_sb")
        nc.sync.dma_start(wexp_sb[:], wexp[:, :])

        def load_coalesced(name, dram, k, inner, eng):
            t = setup.tile([P, k * inner], BF16, name=name)
            eng.dma_start(
                t[:].rearrange("p (k t) -> p k t", k=k),
                dram[:].rearrange("(k p) t -> p k t", p=P),
            )
            return t

        w1m_sb = load_coalesced("w1m_sb", w1m, NK, H, nc.scalar)
        xT_sb = load_coalesced("xT_sb", xT, NK, T, nc.sync)
        l2m_sb = load_coalesced("l2m_sb", l2m, NK, H2, nc.scalar)
        xTi_sb = load_coalesced("xTi_sb", xTi, NK, rows, nc.sync)
        w1h_sb = load_coalesced("w1h_sb", w1h, NK, H, nc.scalar)
        l2h_sb = load_coalesced("l2h_sb", l2h, NK, H2, nc.sync)

        setup_ps_cm = tc.tile_pool(name="setup_ps", bufs=4, space="PSUM")
        setup_ps = setup_ps_cm.__enter__()

        # ---- actM^T = tanh(W1M^T @ xc^T + cbm), bf16 [h1-chunk | j] ----
        actMT = setup.tile([P, NK * T], BF16, name="actMT")
        for hc in range(NK):
            for n0 in (0, TH):
                ps = setup_ps.tile([P, TH], F32, tag="setup")
                for cc in range(NK):
                    nc.tensor.matmul(
                        ps[:],
                        lhsT=w1m_sb[:, cc * H + hc * P:cc * H + (hc + 1) * P],
                        rhs=xT_sb[:, cc * T + n0:cc * T + n0 + TH],
                        start=(cc == 0),
                        stop=(cc == NK - 1),
                    )
                nc.scalar.activation(
                    actMT[:, hc * T + n0:hc * T + n0 + TH], ps[:], TANH,
                    bias=cbm_sb[hc][:],
                )

        # ---- actH^T for this core's rows ----
        actHT = setup.tile([P, NK * rows], BF16, name="actHT")
        for hc in range(NK):
            ps = setup_ps.tile([P, rows], F32, tag="setup")
            for cc in range(NK):
                nc.tensor.matmul(
                    ps[:],
                    lhsT=w1h_sb[:, cc * H + hc * P:cc * H + (hc + 1) * P],
                    rhs=xTi_sb[:, cc * rows:(cc + 1) * rows],
                    start=(cc == 0),
                    stop=(cc == NK - 1),
                )
            nc.scalar.activation(
                actHT[:, hc * rows:(hc + 1) * rows], ps[:], TANH,
                bias=cbh_sb[hc][:],
            )

        # ---- a = AH^T + h2b (fp32), U-side base trig ----
        asb = setup.tile([P, NK * rows], F32, name="asb")
        for hc in range(NK):
            ps = setup_ps.tile([P, rows], F32, tag="setup")
            for kc in range(NK):
                nc.tensor.matmul(
                    ps[:],
                    lhsT=l2h_sb[:, kc * H2 + hc * P:kc * H2 + (hc + 1) * P],
                    rhs=actHT[:, kc * rows:(kc + 1) * rows],
                    start=(kc == 0),
                    stop=(kc == NK - 1),
                )
            nc.vector.tensor_scalar_add(
                asb[:, hc * rows:(hc + 1) * rows], ps[:], h2b_sb[hc][:]
            )

        # U chains in fp32: SCu[k] = [sin(w_k a) | cos(w_k a)], w_k=(2k+1)w0
        HUW = UW // 2  # 384
        qu = setup.tile([P, UW], F32, name="qu")
        SCu = [setup.tile([P, UW], F32, name=f"SCu{k}") for k in range(K)]
        nc.scalar.activation(SCu[0][:, :HUW], asb[:], SIN, scale=W0)
        nc.scalar.activation(SCu[0][:, HUW:], asb[:], SIN, scale=W0,
                             bias=pihalf[:])
        nc.scalar.activation(qu[:, :HUW], asb[:], SIN, scale=2 * W0,
                             bias=pihalf[:])
        nc.vector.tensor_copy(qu[:, HUW:], qu[:, :HUW])
        tu = setup.tile([P, UW], F32, name="tu")
        nc.vector.tensor_tensor(tu[:], qu[:], SCu[0][:], MULT)
        nc.vector.scalar_tensor_tensor(
            SCu[1][:, :HUW], tu[:, :HUW], 2.0, SCu[0][:, :HUW], MULT, ADD)
        nc.vector.scalar_tensor_tensor(
            SCu[1][:, HUW:], tu[:, HUW:], 2.0, SCu[0][:, HUW:], MULT, SUB)
        for k in range(2, K):
            nc.vector.tensor_tensor(tu[:], qu[:], SCu[k - 1][:], MULT)
            nc.vector.scalar_tensor_tensor(
                SCu[k][:], tu[:], 2.0, SCu[k - 2][:], MULT, SUB)
        # scaled bf16 lhsT tiles: USC[k] = (SCu[k] * g_k) .* wexp
        USC = [const.tile([P, UW], BF16, name=f"USC{k}") for k in range(K)]
        for k in range(K):
            nc.vector.scalar_tensor_tensor(
                USC[k][:], SCu[k][:], float(GAMMA[k]), wexp_sb[:], MULT, MULT)

        # ---- B = AM^T (fp32) + V-side base trig, per h2-chunk ----
        Bsb = setup.tile([P, NK * T], F32, name="Bsb")
        VW = 2 * NK * T  # 6144
        HVW = VW // 2    # 3072
        qv = const.tile([P, VW], BF16, name="qv")
        SCv = [const.tile([P, VW], BF16, name=f"SCv{k}") for k in range(K)]
        for hc in range(NK):
            for n0 in (0, TH):
                ps = setup_ps.tile([P, TH], F32, tag="setup")
                for kc in range(NK):
                    nc.tensor.matmul(
                        ps[:],
                        lhsT=l2m_sb[:, kc * H2 + hc * P:kc * H2 + (hc + 1) * P],
                        rhs=actMT[:, kc * T + n0:kc * T + n0 + TH],
                        start=(kc == 0),
                        stop=(kc == NK - 1),
                    )
                nc.vector.tensor_copy(
                    Bsb[:, hc * T + n0:hc * T + n0 + TH], ps[:])
            bs = Bsb[:, hc * T:(hc + 1) * T]
            nc.scalar.activation(
                SCv[0][:, hc * T:(hc + 1) * T], bs, SIN, scale=W0)
            nc.scalar.activation(
                SCv[0][:, HVW + hc * T:HVW + (hc + 1) * T], bs, SIN,
                scale=W0, bias=pihalf[:])
            nc.scalar.activation(
                qv[:, hc * T:(hc + 1) * T], bs, SIN, scale=2 * W0,
                bias=pihalf[:])
        nc.vector.tensor_copy(qv[:, HVW:], qv[:, :HVW])

        # V recurrence in bf16 (concatenated sin|cos, 2 DVE ops per stream)
        tv = setup.tile([P, VW], BF16, name="tv")
        nc.vector.tensor_tensor(tv[:], qv[:], SCv[0][:], MULT)
        nc.vector.scalar_tensor_tensor(
            SCv[1][:, :HVW], tv[:, :HVW], 2.0, SCv[0][:, :HVW], MULT, ADD)
        nc.vector.scalar_tensor_tensor(
            SCv[1][:, HVW:], tv[:, HVW:], 2.0, SCv[0][:, HVW:], MULT, SUB)
        for k in range(2, K):
            nc.vector.tensor_tensor(tv[:], qv[:], SCv[k - 1][:], MULT)
            nc.vector.scalar_tensor_tensor(
                SCv[k][:], tv[:], 2.0, SCv[k - 2][:], MULT, SUB)

        # ---- main contraction: scores = sum_k U_s V_c^T + U_c V_s^T ----
        with tc.tile_pool(name="row_ps", bufs=2, space="PSUM") as row_ps:
            psr = [row_ps.tile([rows, TH], F32, tag="row", name=f"psr{b}")
                   for b in range(2)]
            nmm = K * NK * 2
            idx = 0
            for k in range(K):
                for hc in range(NK):
                    for (u0, v0) in ((0, HVW), (HUW, 0)):  # (sin,cos),(cos,sin)
                        idx += 1
                        for b, n0 in enumerate((0, TH)):
                            nc.tensor.matmul(
                                psr[b][:],
                                lhsT=USC[k][:, u0 + hc * rows:
                                            u0 + (hc + 1) * rows],
                                rhs=SCv[k][:, v0 + hc * T + n0:
                                           v0 + hc * T + n0 + TH],
                                start=(idx == 1),
                                stop=(idx == nmm),
                            )
            ev = const.tile([P, T], F32, name="ev")
            for b, n0 in enumerate((0, TH)):
                nc.vector.tensor_scalar_add(
                    ev[0:rows, n0:n0 + TH], psr[b][:], ob_sb[0:rows, :])
            nc.sync.dma_start(out_rows[:, :], ev[0:rows, :])

        setup_ps_cm.__exit__(None, None, None)
        setup_cm.__exit__(None, None, None)


def _prep_inputs(x, hidLayerFOH, hidLayerFOM, catBias, hid2Layer, hid2Bias,
                 outLayer, outBias, rows=R, ncores=NCORES):
    """Host-side layout prep (reshape/transpose/slice/cast only)."""
    bf = ml_dtypes.bfloat16
    x = np.asarray(x, np.float32)
    xc = x.reshape(T, C)
    wout = np.asarray(outLayer, np.float32).reshape(NK, P).T  # [128, 4]
    wexp = np.tile(np.repeat(wout, rows, axis=1), (1, 2))     # [128, 768]
    cb_all = np.concatenate([
        np.asarray(catBias[H:], np.float32).reshape(NK, P).T,
        np.asarray(catBias[:H], np.float32).reshape(NK, P).T,
        np.asarray(hid2Bias, np.float32).reshape(NK, P).T,
        np.full((P, 1), math.pi / 2, np.float32),
        np.full((P, 1), np.asarray(outBias, np.float32).reshape(()), np.float32),
    ], axis=1)
    common = {
        "xT": np.ascontiguousarray(xc.T).astype(bf),
        "w1m": np.asarray(hidLayerFOM, np.float32).astype(bf),
        "w1h": np.asarray(hidLayerFOH, np.float32).astype(bf),
        "l2m": np.asarray(hid2Layer, np.float32)[H:].astype(bf),
        "l2h": np.asarray(hid2Layer, np.float32)[:H].astype(bf),
        "cb": np.ascontiguousarray(cb_all),
        "wexp": np.ascontiguousarray(wexp),
    }
    in_maps = []
    for c in range(ncores):
        m = dict(common)
        m["xTi"] = np.ascontiguousarray(
            xc[c * rows:(c + 1) * rows].T).astype(bf)
        in_maps.append(m)
    return in_maps


def kernel(x, hidLayerFOH, hidLayerFOM, catBias, hid2Layer, hid2Bias,
           outLayer, outBias, _trace=False):
    in_maps = _prep_inputs(x, hidLayerFOH, hidLayerFOM, catBias,
                           hid2Layer, hid2Bias, outLayer, outBias)
    nc = build_nc(R)
    res = run_bass_kernel_spmd(nc, in_maps, core_ids=list(range(NCORES)),
                               trace=_trace)
    out = np.concatenate([res.results[c]["out_rows"] for c in range(NCORES)], 0)
    if _trace:
        kernel.last_results = res
    return out.astype(np.float32)


# revision 3
# speedup vs baseline: 4.3194x; 1.5133x over previous
"""Trainium2 Bass kernel for nn_ConcatHeadModule (pairwise concat-head scorer).

Math (reference):
    xc   = x.reshape(T, 2L)
    actH = tanh(xc @ W1H + cbH);  actM = tanh(xc @ W1M + cbM)
    AH   = actH @ L2H;            AM   = actM @ L2M
    scores[i,j] = sum_h wOut[h]*tanh(AH[i,h] + AM[j,h] + h2b[h]) + outBias

Instead of evaluating the T*T*H2 pairwise tanh on ScalarE (the baseline's
bottleneck: ~250us of ACTIVATE per core), tanh is expanded in an odd
harmonic sine series fitted offline:

    tanh(s) ~= sum_k g_k * sin((2k-1)*w0*s),   k = 1..K

Each term is separable across the pairwise sum s = a_i + B_j:

    sin(w(a+B)) = sin(wa)cos(wB) + cos(wa)sin(wB)

so the score matrix becomes a single TensorE contraction over (k, h):

    scores[i,j] = sum_k sum_h [g_k*w_h*sin(w_k a)]*cos(w_k B)
                            + [g_k*w_h*cos(w_k a)]*sin(w_k B)

Only the base streams sin(w0 v), cos(w0 v), cos(2 w0 v) are evaluated with
ScalarE's Sin table (all arguments stay inside its valid [-pi, pi] domain,
read straight out of the AM/AH PSUM tiles); higher harmonics come from the
3-term recurrence  s_{k+1} = 2*cos(2 w0 v)*s_k - s_{k-1}  on the vector
engine, in bf16 tensor_tensor ops (2x perf mode), with the sin and cos
chains sharing tiles. Sharding: rows of the score grid, 96 per core.

All shapes are hardcoded (T=768, 2L=512, HID=512, HID2=512, 8 cores).
"""

import os
import sys

for _p in ("/root/.axon_site", "/root/.axon_site/_ro/trn_rl_repo", "/opt/trn_rl_repo"):
    if os.path.isdir(_p) and _p not in sys.path:
        sys.path.append(_p)

import math

import ml_dtypes
import numpy as np

import concourse.bass as bass
import concourse.mybir as mybir
import concourse.tile as tile
from concourse import bacc
from concourse.bass_utils import run_bass_kernel_spmd

F32 = mybir.dt.float32
BF16 = mybir.dt.bfloat16
TANH = mybir.ActivationFunctionType.Tanh
SIN = mybir.ActivationFunctionType.Sin
IDENT = mybir.ActivationFunctionType.Identity
MULT = mybir.AluOpType.mult
ADD = mybir.AluOpType.add
SUB = mybir.AluOpType.subtract

T = 768          # tokens
C = 512          # 2 * LDIMS (concat lstm state)
H = 512          # hidden1
H2 = 512         # hidden2
NCORES = 8
R = T // NCORES  # score rows per core
P = 128          # partitions
NK = 4           # 128-chunks in C/H/H2
TH = T // 2      # 384, psum half-width for j

# Odd-harmonic sine expansion of tanh: tanh(s) ~= sum g[k] sin((2k+1) W0 s)
W0 = 0.3340
GAMMA = [1.216696, 0.289241, 0.097495, 0.033947, 0.013563]
K = len(GAMMA)


def build_nc(rows: int = R) -> bass.Bass:
    nc = bacc.Bacc("TRN2", target_bir_lowering=False, num_devices=NCORES)

    xT = nc.dram_tensor("xT", [C, T], BF16, kind="ExternalInput")
    xTi = nc.dram_tensor("xTi", [C, rows], BF16, kind="ExternalInput")
    w1m = nc.dram_tensor("w1m", [C, H], BF16, kind="ExternalInput")
    w1h = nc.dram_tensor("w1h", [C, H], BF16, kind="ExternalInput")
    l2m = nc.dram_tensor("l2m", [H, H2], BF16, kind="ExternalInput")
    l2h = nc.dram_tensor("l2h", [H, H2], BF16, kind="ExternalInput")
    # cb columns: cbm[4], cbh[4], ub1[4]=W0*h2b, ub2[4]=W0*h2b+pi/2,
    #             ub3[4]=2*W0*h2b+pi/2, pihalf[1], ob[1]
    cb = nc.dram_tensor("cb", [P, 5 * NK + 2], F32, kind="ExternalInput")
    wexp = nc.dram_tensor("wexp", [P, 2 * NK * R], BF16, kind="ExternalInput")
    out_rows = nc.dram_tensor("out_rows", [rows, T], F32, kind="ExternalOutput")

    with tile.TileContext(nc) as tc:
        _emit(tc, locals(), rows)
    nc.compile()
    return nc


def _emit(tc: tile.TileContext, io, rows: int):
    nc = tc.nc
    xT, xTi, w1m, w1h = io["xT"], io["xTi"], io["w1m"], io["w1h"]
    l2m, l2h, cb, wexp = io["l2m"], io["l2h"], io["cb"], io["wexp"]
    out_rows = io["out_rows"]
    UW = 2 * NK * rows   # 768: [sin 4hc | cos 4hc] U tile width
    HUW = UW // 2        # 384
    VB = 2 * T           # 1536: per-hc V block [sin 768 | cos 768]
    VW = NK * VB         # 6144
    HVW = VW // 2        # 3072

    with tc.tile_pool(name="const", bufs=1) as const:
        setup_cm = tc.tile_pool(name="setup_sb", bufs=1)
        setup = setup_cm.__enter__()

        # dummy activation first: forces the ACT table load (~2.7us) to
        # overlap the input DMAs instead of stalling the first tanh
        scratch = setup.tile([P, 8], F32, name="scratch")
        nc.vector.memset(scratch[:, 0:4], 0.0)
        nc.scalar.activation(scratch[:, 4:8], scratch[:, 0:4], SIN)

        # ---- input DMAs (few, coalesced; alternate the two HWDGE queues) ----
        cb_all = const.tile([P, 5 * NK + 2], F32, name="cb_all")
        nc.sync.dma_start(cb_all[:], cb[:, :])
        cbm_sb = [cb_all[:, k:k + 1] for k in range(NK)]
        cbh_sb = [cb_all[:, NK + k:NK + k + 1] for k in range(NK)]
        ub_sb = [[cb_all[:, (1 + v) * NK + k:(1 + v) * NK + k + 1]
                  for k in range(NK)] for v in (1, 2, 3)]
        pihalf = cb_all[:, 5 * NK:5 * NK + 1]
        ob_sb = cb_all[:, 5 * NK + 1:5 * NK + 2]
        wexp_sb = const.tile([P, UW], BF16, name="wexp_sb")
        nc.sync.dma_start(wexp_sb[:], wexp[:, :])

        def load_coalesced(name, dram, k, inner, eng):
            t = setup.tile([P, k * inner], BF16, name=name)
            eng.dma_start(
                t[:].rearrange("p (k t) -> p k t", k=k),
                dram[:].rearrange("(k p) t -> p k t", p=P),
            )
            return t

        w1m_sb = load_coalesced("w1m_sb", w1m, NK, H, nc.scalar)
        xT_sb = load_coalesced("xT_sb", xT, NK, T, nc.sync)
        xTi_sb = load_coalesced("xTi_sb", xTi, NK, rows, nc.sync)
        w1h_sb = load_coalesced("w1h_sb", w1h, NK, H, nc.scalar)
        l2m_sb = load_coalesced("l2m_sb", l2m, NK, H2, nc.scalar)
        l2h_sb = load_coalesced("l2h_sb", l2h, NK, H2, nc.sync)

        setup_ps_cm = tc.tile_pool(name="setup_ps", bufs=4, space="PSUM")
        setup_ps = setup_ps_cm.__enter__()

        # ---- actM^T = tanh(W1M^T @ xc^T + cbm), bf16 [h1-chunk | j] ----
        actMT = setup.tile([P, NK * T], BF16, name="actMT")
        for hc in range(NK):
            for n0 in (0, TH):
                ps = setup_ps.tile([P, TH], F32, tag="setup")
                for cc in range(NK):
                    nc.tensor.matmul(
                        ps[:],
                        lhsT=w1m_sb[:, cc * H + hc * P:cc * H + (hc + 1) * P],
                        rhs=xT_sb[:, cc * T + n0:cc * T + n0 + TH],
                        start=(cc == 0),
                        stop=(cc == NK - 1),
                    )
                nc.scalar.activation(
                    actMT[:, hc * T + n0:hc * T + n0 + TH], ps[:], TANH,
                    bias=cbm_sb[hc][:],
                )

        # ---- H side: actH^T, then a = AH^T + h2b folded into ACT biases ----
        actHT = setup.tile([P, NK * rows], BF16, name="actHT")
        for hc in range(NK):
            ps = setup_ps.tile([P, rows], F32, tag="setup")
            for cc in range(NK):
                nc.tensor.matmul(
                    ps[:],
                    lhsT=w1h_sb[:, cc * H + hc * P:cc * H + (hc + 1) * P],
                    rhs=xTi_sb[:, cc * rows:(cc + 1) * rows],
                    start=(cc == 0),
                    stop=(cc == NK - 1),
                )
            nc.scalar.activation(
                actHT[:, hc * rows:(hc + 1) * rows], ps[:], TANH,
                bias=cbh_sb[hc][:],
            )

        # U base streams straight from the AH psum:
        #   sin(W0 a) = Sin(W0*AH + W0*h2b), cos via +pi/2, q = cos(2 W0 a)
        u1raw = setup.tile([P, UW], BF16, name="u1raw")
        qu = setup.tile([P, HUW], BF16, name="qu")
        for hc in range(NK):
            ps = setup_ps.tile([P, rows], F32, tag="setup")
            for kc in range(NK):
                nc.tensor.matmul(
                    ps[:],
                    lhsT=l2h_sb[:, kc * H2 + hc * P:kc * H2 + (hc + 1) * P],
                    rhs=actHT[:, kc * rows:(kc + 1) * rows],
                    start=(kc == 0),
                    stop=(kc == NK - 1),
                )
            sl = slice(hc * rows, (hc + 1) * rows)
            nc.scalar.activation(u1raw[:, sl], ps[:], SIN,
                                 scale=W0, bias=ub_sb[0][hc][:])
            nc.scalar.activation(u1raw[:, HUW + hc * rows:HUW + (hc + 1) * rows],
                                 ps[:], SIN, scale=W0, bias=ub_sb[1][hc][:])
            nc.scalar.activation(qu[:, sl], ps[:], SIN,
                                 scale=2 * W0, bias=ub_sb[2][hc][:])

        # U chains (bf16, w-scaled seeds so wOut propagates for free)
        SCu = [setup.tile([P, UW], BF16, name=f"SCu{k}") for k in range(K)]
        nc.vector.tensor_tensor(SCu[0][:], u1raw[:], wexp_sb[:], MULT)
        mu = setup.tile([P, UW], BF16, name="mu")
        mpmu = setup.tile([P, UW], BF16, name="mpmu")
        nc.vector.tensor_scalar(mu[:, :HUW], qu[:], 2.0, None, MULT)
        nc.vector.tensor_scalar(mu[:, HUW:], qu[:], 2.0, None, MULT)
        nc.vector.tensor_scalar(mpmu[:, :HUW], qu[:], 2.0, 1.0, MULT, ADD)
        nc.vector.tensor_scalar(mpmu[:, HUW:], qu[:], 2.0, -1.0, MULT, ADD)
        nc.vector.tensor_tensor(SCu[1][:], mpmu[:], SCu[0][:], MULT)
        tu = setup.tile([P, UW], BF16, name="tu")
        for k in range(2, K):
            nc.vector.tensor_tensor(tu[:], mu[:], SCu[k - 1][:], MULT)
            nc.vector.tensor_tensor(SCu[k][:], tu[:], SCu[k - 2][:], SUB)
        USC = [const.tile([P, UW], BF16, name=f"USC{k}") for k in range(K)]
        for k in range(K):
            nc.vector.tensor_scalar(USC[k][:], SCu[k][:], float(GAMMA[k]),
                                    None, MULT)

        # ---- V side: AM^T psum -> base trig directly, hc-major layout ----
        # SCv[k] blocks: [hc0: sin(768)|cos(768)] [hc1: ...] ...
        qv = const.tile([P, HVW], BF16, name="qv")  # hc-major cos(2 W0 B)
        SCv = [const.tile([P, VW], BF16, name=f"SCv{k}") for k in range(K)]
        for hc in range(NK):
            for n0 in (0, TH):
                ps = setup_ps.tile([P, TH], F32, tag="setup")
                for kc in range(NK):
                    nc.tensor.matmul(
                        ps[:],
                        lhsT=l2m_sb[:, kc * H2 + hc * P:kc * H2 + (hc + 1) * P],
                        rhs=actMT[:, kc * T + n0:kc * T + n0 + TH],
                        start=(kc == 0),
                        stop=(kc == NK - 1),
                    )
                nc.scalar.activation(
                    SCv[0][:, hc * VB + n0:hc * VB + n0 + TH], ps[:], SIN,
                    scale=W0)
                nc.scalar.activation(
                    SCv[0][:, hc * VB + T + n0:hc * VB + T + n0 + TH], ps[:],
                    SIN, scale=W0, bias=pihalf[:])
                nc.scalar.activation(
                    qv[:, hc * T + n0:hc * T + n0 + TH], ps[:], SIN,
                    scale=2 * W0, bias=pihalf[:])

        mv = const.tile([P, VW], BF16, name="mv")
        mpmv = setup.tile([P, VW], BF16, name="mpmv")
        for hc in range(NK):
            qb = qv[:, hc * T:(hc + 1) * T]
            nc.vector.tensor_scalar(mv[:, hc * VB:hc * VB + T], qb, 2.0,
                                    None, MULT)
            nc.vector.tensor_scalar(mv[:, hc * VB + T:(hc + 1) * VB], qb, 2.0,
                                    None, MULT)
            nc.vector.tensor_scalar(mpmv[:, hc * VB:hc * VB + T], qb, 2.0,
                                    1.0, MULT, ADD)
            nc.vector.tensor_scalar(mpmv[:, hc * VB + T:(hc + 1) * VB], qb,
                                    2.0, -1.0, MULT, ADD)
        # recurrence emitted in 2-hc halves so TensorE can start consuming
        # stream k while the other half of k is still on the DVE
        tv = setup.tile([P, VW], BF16, name="tv")
        for half in (0, 1):
            hs = slice(half * HVW, (half + 1) * HVW)
            nc.vector.tensor_tensor(SCv[1][:, hs], mpmv[:, hs], SCv[0][:, hs],
                                    MULT)
        for k in range(2, K):
            for half in (0, 1):
                hs = slice(half * HVW, (half + 1) * HVW)
                nc.vector.tensor_tensor(tv[:, hs], mv[:, hs],
                                        SCv[k - 1][:, hs], MULT)
                nc.vector.tensor_tensor(SCv[k][:, hs], tv[:, hs],
                                        SCv[k - 2][:, hs], SUB)

        # ---- main contraction: scores = sum_k U_s V_c^T + U_c V_s^T ----
        with tc.tile_pool(name="row_ps", bufs=2, space="PSUM") as row_ps:
            psr = [row_ps.tile([rows, TH], F32, tag="row", name=f"psr{b}")
                   for b in range(2)]
            nmm = K * NK * 2
            idx = 0
            for k in range(K):
                for hc in range(NK):
                    # (U sin, V cos), (U cos, V sin)
                    for (u0, v0) in ((0, hc * VB + T), (HUW, hc * VB)):
                        idx += 1
                        for b, n0 in enumerate((0, TH)):
                            nc.tensor.matmul(
                                psr[b][:],
                                lhsT=USC[k][:, u0 + hc * rows:
                                            u0 + (hc + 1) * rows],
                                rhs=SCv[k][:, v0 + n0:v0 + n0 + TH],
                                start=(idx == 1),
                                stop=(idx == nmm),
                            )
            ev = const.tile([P, T], F32, name="ev")
            for b, n0 in enumerate((0, TH)):
                nc.scalar.activation(ev[0:rows, n0:n0 + TH], psr[b][:], IDENT,
                                     bias=ob_sb[0:rows, :])
            nc.sync.dma_start(out_rows[:, :], ev[0:rows, :])

        setup_ps_cm.__exit__(None, None, None)
        setup_cm.__exit__(None, None, None)


def _prep_inputs(x, hidLayerFOH, hidLayerFOM, catBias, hid2Layer, hid2Bias,
                 outLayer, outBias, rows=R, ncores=NCORES):
    """Host-side layout prep (reshape/transpose/slice/cast only)."""
    bf = ml_dtypes.bfloat16
    x = np.asarray(x, np.float32)
    xc = x.reshape(T, C)
    wout = np.asarray(outLayer, np.float32).reshape(NK, P).T  # [128, 4]
    wexp = np.tile(np.repeat(wout, rows, axis=1), (1, 2))     # [128, 768]
    h2b = np.asarray(hid2Bias, np.float32).reshape(NK, P).T
    cb_all = np.concatenate([
        np.asarray(catBias[H:], np.float32).reshape(NK, P).T,
        np.asarray(catBias[:H], np.float32).reshape(NK, P).T,
        W0 * h2b,
        W0 * h2b + math.pi / 2,
        2 * W0 * h2b + math.pi / 2,
        np.full((P, 1), math.pi / 2, np.float32),
        np.full((P, 1), np.asarray(outBias, np.float32).reshape(()), np.float32),
    ], axis=1).astype(np.float32)
    common = {
        "xT": np.ascontiguousarray(xc.T).astype(bf),
        "w1m": np.asarray(hidLayerFOM, np.float32).astype(bf),
        "w1h": np.asarray(hidLayerFOH, np.float32).astype(bf),
        "l2m": np.asarray(hid2Layer, np.float32)[H:].astype(bf),
        "l2h": np.asarray(hid2Layer, np.float32)[:H].astype(bf),
        "cb": np.ascontiguousarray(cb_all),
        "wexp": np.ascontiguousarray(wexp).astype(bf),
    }
    in_maps = []
    for c in range(ncores):
        m = dict(common)
        m["xTi"] = np.ascontiguousarray(
            xc[c * rows:(c + 1) * rows].T).astype(bf)
        in_maps.append(m)
    return in_maps


def kernel(x, hidLayerFOH, hidLayerFOM, catBias, hid2Layer, hid2Bias,
           outLayer, outBias, _trace=False):
    in_maps = _prep_inputs(x, hidLayerFOH, hidLayerFOM, catBias,
                           hid2Layer, hid2Bias, outLayer, outBias)
    nc = build_nc(R)
    res = run_bass_kernel_spmd(nc, in_maps, core_ids=list(range(NCORES)),
                               trace=_trace)
    out = np.concatenate([res.results[c]["out_rows"] for c in range(NCORES)], 0)
    if _trace:
        kernel.last_results = res
    return out.astype(np.float32)
